# revision 2
# baseline (speedup 1.0000x reference)
"""Exponential smoothing (per-channel EMA over time) on 8 Trainium2 cores.

  s_0 = x_0 ; s_t = a * x_t + (1 - a) * s_{t-1},  a = sigmoid(alpha)  (per channel)

Full shapes: x (16, 4096, 512) f32, alpha (1, 1, 512) f32 -> out (16, 4096, 512).

The kernel is DMA-bandwidth-bound (per-core DMA bus ~360 GB/s shared by loads
and stores), so the design minimizes HBM bytes and on-device data motion:

  * Host preps each core's shard as time-major bf16: x[b] -> (D, T) bf16.
    The 2e-2 global-rel-err budget dwarfs bf16's 2^-9 rounding, and the
    EMA is a convex combination so input rounding does not amplify.
    Halves DMA traffic vs f32 and removes the on-device transposes
    (channels land directly on partitions, time on the free axis).
  * Device per (batch, 128-channel chunk) chain:
      1. DMA-load x^T tile [128, TC] bf16 (contiguous TC*2 B per partition).
      2. u = a * x on the scalar engine (per-partition scale, fuses the
         bf16 -> f32 upcast).
      3. Hardware scan (TensorTensorScanArith) on the vector engine:
         state = w * state + u, w = 1 - a = sigmoid(-alpha). Scan state is
         fp32 internally regardless of operand dtype and the output is
         downcast to bf16 on write -- no separate downcast pass. Chunks
         chain via initial = previous chunk's last column; chunk 0 uses
         initial = x_0 (bf16), making s_0 = w*x_0 + a*x_0 = x_0.
      4. DMA-store y^T tile [128, TC] bf16 on the GpSimd SWDGE ring.
  * Host transposes back to (B, T, D) and upcasts to f32.
"""

from contextlib import ExitStack

import ml_dtypes
import numpy as np

import concourse.tile as tile
from concourse import bacc, mybir
from concourse.bass_utils import run_bass_kernel_spmd

B, T, D = 16, 4096, 512
NCORES = 8
BL = B // NCORES   # batches per core
P = 128            # partitions
TC = 2048          # time chunk per pipeline iteration
ND = D // P        # channel chunks of 128
NTC = T // TC

FP32 = mybir.dt.float32
BF16 = mybir.dt.bfloat16
BF16_NP = ml_dtypes.bfloat16


def build_program(bl: int = BL, t: int = T) -> bacc.Bacc:
    """Build the per-core Bass program (same NEFF for all 8 cores)."""
    ntc = t // TC
    nc = bacc.Bacc(
        "TRN2",
        target_bir_lowering=False,
        debug=False,
        enable_asserts=False,
        num_devices=NCORES,
    )
    x = nc.dram_tensor("xt", (bl, D, t), BF16, kind="ExternalInput").ap()
    alpha = nc.dram_tensor("alpha", (1, 1, D), FP32, kind="ExternalInput").ap()
    y = nc.dram_tensor("yt", (bl, D, t), BF16, kind="ExternalOutput").ap()

    with tile.TileContext(nc) as tc, ExitStack() as ctx:
        const_pool = ctx.enter_context(tc.tile_pool(name="const", bufs=1))
        x_pool = ctx.enter_context(tc.tile_pool(name="x", bufs=6))
        u_pool = ctx.enter_context(tc.tile_pool(name="u", bufs=4))
        y_pool = ctx.enter_context(tc.tile_pool(name="y", bufs=12))

        # alpha (1,1,512) -> (128, ND) tile: channel d = j*128 + p
        alpha_sb = const_pool.tile([P, ND], FP32)
        nc.sync.dma_start(alpha_sb[:], alpha.rearrange("o u (j p) -> (o u p) j", p=P))
        a_sb = const_pool.tile([P, ND], FP32)  # a = sigmoid(alpha)
        nc.scalar.activation(a_sb[:], alpha_sb[:], mybir.ActivationFunctionType.Sigmoid)
        w_sb = const_pool.tile([P, ND], FP32)  # w = 1 - a = sigmoid(-alpha)
        nc.scalar.activation(
            w_sb[:], alpha_sb[:], mybir.ActivationFunctionType.Sigmoid, scale=-1.0
        )

        # Per-channel-chunk decay tiles broadcast along the time axis
        # (scan data0 must be a full [P, TC] operand).
        ones = const_pool.tile([P, TC], FP32)
        nc.vector.memset(ones[:], 1.0)
        wbs = []
        for j in range(ND):
            wt = const_pool.tile([P, TC], FP32, tag=f"wb{j}")
            nc.vector.tensor_scalar_mul(wt[:], ones[:], w_sb[:, j : j + 1])
            wbs.append(wt)

        # 2*ND = 8 independent scan chains keep the pipeline full; only the
        # per-chain chunk order is serialized (via `initial`).
        y_prev = [[None] * ND for _ in range(bl)]
        x_first = [[None] * ND for _ in range(bl)]
        for tci in range(ntc):
            t0 = tci * TC
            for b in range(bl):
                for j in range(ND):
                    xc = x_pool.tile([P, TC], BF16, tag="x")
                    nc.sync.dma_start(xc[:], x[b, j * P : (j + 1) * P, t0 : t0 + TC])
                    if tci == 0:
                        x_first[b][j] = xc
                    u = u_pool.tile([P, TC], FP32, tag="u")
                    nc.scalar.mul(u[:], xc[:], a_sb[:, j : j + 1])
                    yc = y_pool.tile([P, TC], BF16, tag="y")
                    init = (
                        x_first[b][j][:, 0:1]
                        if tci == 0
                        else y_prev[b][j][:, TC - 1 : TC]
                    )
                    nc.vector.tensor_tensor_scan(
                        yc[:],
                        wbs[j][:],
                        u[:],
                        init,
                        mybir.AluOpType.mult,
                        mybir.AluOpType.add,
                    )
                    y_prev[b][j] = yc
                    nc.gpsimd.dma_start(y[b, j * P : (j + 1) * P, t0 : t0 + TC], yc[:])

    nc.compile()
    return nc


_prog = None


def _get_prog():
    global _prog
    if _prog is None:
        _prog = build_program()
    return _prog


def make_in_maps(x, alpha):
    """Per-core inputs: time-major bf16 shard of x + replicated alpha."""
    x = np.asarray(x)
    alpha = np.ascontiguousarray(np.asarray(alpha, dtype=np.float32))
    assert x.shape == (B, T, D) and alpha.shape == (1, 1, D)
    xt = np.ascontiguousarray(x.transpose(0, 2, 1)).astype(BF16_NP)  # (B, D, T)
    return [
        {"xt": np.ascontiguousarray(xt[i * BL : (i + 1) * BL]), "alpha": alpha}
        for i in range(NCORES)
    ]


def gather(results):
    """(NCORES, BL, D, T) bf16 shards -> (B, T, D) f32."""
    yt = np.concatenate([r["yt"] for r in results], axis=0)  # (B, D, T) bf16
    return np.ascontiguousarray(yt.transpose(0, 2, 1)).astype(np.float32)


def kernel(x, alpha):
    res = run_bass_kernel_spmd(
        _get_prog(), make_in_maps(x, alpha), core_ids=list(range(NCORES))
    )
    return gather(res.results)
